# revision 22
# baseline (speedup 1.0000x reference)
"""NetVLAD Trainium2 kernel (data-parallel over batch across 8 NeuronCores).

Math per image (x: [D=512, P=4096], conv_w: [K=64, D], centroids c: [K, D]):
  xhat = x / ||x||_2(over D, per pixel)
  logitsT[p, k] = sum_d xhat[d, p] * conv_w[k, d]
  a = softmax_k(logitsT)            (|logits| <= ||w_k|| ~ 1.3 -> no max-sub)
  vlad[k, d] = sum_p a[p, k] * xhat[d, p] - (sum_p a[p, k]) * c[k, d]
  out = l2norm_global(l2norm_rows(vlad))

Folding (avoids materializing xhat):
  e[p,k]  = exp(invnorm[p] * raw_logit[p,k])     raw logits from RAW x
  e'[p,k] = e[p,k] * invnorm[p] / S[p]           S = sum_k e
  vlad    = sum_p e'[p,k] * x[p,d]               (matmul2)
  A[k]    = sum_p e'[p,k] * norm[p]              (2nd matmul, rhs = norms)

Implementation notes (bf16 on-chip via SWDGE cast-DMA):
  - walrus allows at most ONE sem wait per instruction, so the dataflow is
    arranged so every instruction depends on at most one foreign engine
    tick that its engine has not yet observed. PE `ldweights` ops are used
    as pure wait-absorbers where needed.
  - Per p-chunk the PE loads the x chunk once as stationary and issues
    matmul (logits, fp32 PSUM) then transpose-mode matmul vs identity
    (xT chunk, bf16 PSUM). The PSUM->SBUF copy and the sumsq for the pixel
    norms are done by ONE engine per chunk (ACT or DVE, balanced).
  - Per-pixel softmax pipeline batched per 8-chunk group, all on partitions.
"""

import numpy as np
import ml_dtypes

N, D, HH, WW, K = 32, 512, 64, 64, 64
P = HH * WW            # 4096
NCORES = 8
NPER = N // NCORES     # 4 images per core
DC = D // 128          # 4 d-chunks
PC = P // 128          # 32 p-chunks
GRP = 8                # p-chunks per softmax batch group
NG = PC // GRP

TRACE = False          # test.py sets this for profiled runs
IMGS = NPER            # debug knob: build fewer images
STAGE = 5              # 0 prod,1 +copy/ssq,2 +norms/exp,3 +softmax,4 +mm2,5 full
_CACHE = {}

# chunks handled by ACT (copy+sumsq); rest on DVE. pc0 must be ACT so the
# recycled sumsq tile's first writer needs only its PE wait.
ACT_PCS_PER_GRP = 3    # pcs g*GRP + {0..ACT_PCS_PER_GRP-1} go to ACT


def _on_act(pc):
    return pc % GRP < ACT_PCS_PER_GRP


def _build():
    import concourse.bass as bass
    import concourse.bacc as bacc
    import concourse.tile as tile
    from concourse import mybir
    import concourse.bass_isa as bass_isa

    f32 = mybir.dt.float32
    bf16 = mybir.dt.bfloat16
    FT = mybir.ActivationFunctionType
    ALU = mybir.AluOpType

    nc = bacc.Bacc()
    x_dram = nc.declare_dram_parameter("x", [NPER, D, P], f32, isOutput=False)
    wt_dram = nc.declare_dram_parameter("conv_wt", [D, K], bf16, isOutput=False)
    id_dram = nc.declare_dram_parameter("ident", [128, 128], bf16, isOutput=False)
    c_dram = nc.declare_dram_parameter("cent", [K, D], f32, isOutput=False)
    y_dram = nc.declare_dram_parameter("y", [NPER, K * D], f32, isOutput=True)

    with tile.TileContext(nc) as tc:
        with (
            tc.tile_pool(name="consts", bufs=1) as consts,
            tc.tile_pool(name="xpool", bufs=2) as xpool,
            tc.tile_pool(name="xtpool", bufs=2) as xtpool,
            tc.tile_pool(name="epool", bufs=2) as epool,
            tc.tile_pool(name="stats", bufs=2) as stats,
            tc.tile_pool(name="dumps", bufs=1) as dumps,
            tc.tile_pool(name="epi", bufs=2) as epi,
            tc.tile_pool(name="ps_xt", bufs=2, space="PSUM") as ps_xt,
            tc.tile_pool(name="ps_a", bufs=2, space="PSUM") as ps_a,
            tc.tile_pool(name="ps_log", bufs=2, space="PSUM") as ps_log,
            tc.tile_pool(name="ps_vlad", bufs=2, space="PSUM") as ps_vlad,
        ):
            # ---- constants ----
            wt_sb = consts.tile([128, DC, K], bf16)   # conv_w^T chunked
            nc.gpsimd.dma_start(
                out=wt_sb[:], in_=wt_dram[:].rearrange("(dc dp) k -> dp dc k", dp=128)
            )
            ident = consts.tile([128, 128], bf16)
            nc.gpsimd.dma_start(out=ident[:], in_=id_dram[:])
            cent_sb = consts.tile([K, D], f32)
            nc.gpsimd.dma_start(out=cent_sb[:], in_=c_dram[:])

            act_dump = dumps.tile([128, 512], bf16)
            dve_dump = dumps.tile([128, 512], bf16)
            epi_dump = dumps.tile([K, D], f32)

            for nn in range(IMGS):
                n = nn % NPER
                # ---- load + cast fp32 -> bf16 (SWDGE) ----
                x_bf = xpool.tile([128, DC, P], bf16)
                xn = x_dram[n].rearrange("(dc dp) p -> dp dc p", dp=128)
                nc.gpsimd.dma_start(out=x_bf[:], in_=xn[:])

                sumsq = stats.tile([128, PC], f32)
                invnorm = stats.tile([128, PC], f32)
                norm_bf = stats.tile([128, PC], bf16)
                e_sb = epool.tile([128, PC, K], bf16)
                ep_sb = epool.tile([128, PC, K], bf16)
                S = stats.tile([128, NG, GRP], f32)
                invS = stats.tile([128, PC], f32)
                factor = stats.tile([128, PC, 1], bf16)
                xt_sb = xtpool.tile([128, PC, 512], bf16)
                vlad_acc = epi.tile([K, D], f32)
                a_acc = epi.tile([K, 1], f32)

                # PE wait-absorber: take the x-load wait on a standalone
                # LDWEIGHTS so the first real matmul only carries its
                # pool-recycle wait.
                nc.tensor.ldweights(weights=x_bf[:, 0, 0:128])

                def mm2_group(g):
                    # absorber: observe ACT's xt_sb copies for this group
                    nc.tensor.ldweights(
                        weights=xt_sb[:, g * GRP + ACT_PCS_PER_GRP - 1, 0:128]
                    )
                    # closed per-group PSUM accumulation: the PE may not
                    # switch into transpose mode while a group is open, so
                    # each group's 16 matmuls are contiguous and partials
                    # are accumulated into SBUF by the DVE.
                    vlad_ps = ps_vlad.tile([K, D], f32, tag="vlad_ps")
                    a_ps = ps_a.tile([K, 1], f32, tag="a_ps")
                    for sub in range(GRP):
                        pc = g * GRP + sub
                        nc.tensor.matmul(
                            vlad_ps[:],
                            lhsT=ep_sb[:, pc],
                            rhs=xt_sb[:, pc],
                            start=(sub == 0),
                            stop=(sub == GRP - 1),
                        )
                        nc.tensor.matmul(
                            a_ps[:],
                            lhsT=ep_sb[:, pc],
                            rhs=norm_bf[:, pc:pc + 1],
                            start=(sub == 0),
                            stop=(sub == GRP - 1),
                        )
                    if g == 0:
                        nc.vector.tensor_copy(vlad_acc[:], vlad_ps[:])
                        nc.vector.tensor_copy(a_acc[:], a_ps[:])
                    else:
                        nc.vector.tensor_tensor(
                            out=vlad_acc[:], in0=vlad_acc[:], in1=vlad_ps[:],
                            op=ALU.add,
                        )
                        nc.vector.tensor_tensor(
                            out=a_acc[:], in0=a_acc[:], in1=a_ps[:], op=ALU.add,
                        )

                for g in range(NG if STAGE >= 0 else 0):
                    logT = ps_log.tile([128, GRP, K], f32, tag="logT")
                    for sub in range(GRP):
                        pc = g * GRP + sub
                        # ---- PE: logits + transpose, shared stationary ----
                        xtp = ps_xt.tile([128, 512], bf16, tag="xtp")
                        for dc in range(DC):
                            xchunk = x_bf[:, dc, pc * 128:(pc + 1) * 128]
                            nc.tensor.matmul(
                                logT[:, sub],
                                lhsT=xchunk,
                                rhs=wt_sb[:, dc],
                                start=(dc == 0),
                                stop=(dc == DC - 1),
                            )
                        for dc in range(DC):
                            xchunk = x_bf[:, dc, pc * 128:(pc + 1) * 128]
                            nc.tensor.transpose(
                                xtp[:, dc * 128:(dc + 1) * 128], xchunk, ident[:]
                            )
                        # ---- copy xT PSUM -> SBUF + sumsq (one engine) ----
                        if STAGE < 1:
                            continue
                        nc.vector.tensor_copy(xt_sb[:, pc], xtp[:])
                        nc.scalar.activation(
                            act_dump[:], xtp[:], FT.Square,
                            accum_out=sumsq[:, pc:pc + 1],
                        )

                    # ---- norms for this group ----
                    if STAGE < 2:
                        continue
                    gs = slice(g * GRP, (g + 1) * GRP)
                    norm_g = stats.tile([128, GRP], f32, tag="norm_g")
                    nc.scalar.activation(norm_g[:], sumsq[:, gs], FT.Sqrt)
                    nc.vector.reciprocal(invnorm[:, gs], norm_g[:])
                    nc.vector.tensor_copy(norm_bf[:, gs], norm_g[:])

                    # ---- exp with per-pixel temperature ----
                    for sub in range(GRP):
                        pc = g * GRP + sub
                        nc.scalar.activation(
                            e_sb[:, pc], logT[:, sub], FT.Exp,
                            scale=invnorm[:, pc:pc + 1],
                        )
                    # ---- softmax denominator + folded scale ----
                    if STAGE < 3:
                        continue
                    nc.vector.reduce_sum(
                        S[:, g], e_sb[:, gs], axis=mybir.AxisListType.X
                    )
                    nc.vector.reciprocal(invS[:, gs], S[:, g])
                    nc.vector.tensor_tensor(
                        out=factor[:, gs, 0], in0=invS[:, gs],
                        in1=invnorm[:, gs], op=ALU.mult,
                    )
                    nc.vector.tensor_tensor(
                        out=ep_sb[:, gs],
                        in0=e_sb[:, gs],
                        in1=factor[:, gs].to_broadcast([128, GRP, K]),
                        op=ALU.mult,
                    )
                    # matmul2, software-pipelined one group behind
                    if STAGE >= 4 and g >= 1:
                        mm2_group(g - 1)
                if STAGE >= 4:
                    mm2_group(NG - 1)

                # ---- epilogue ----
                if STAGE < 5:
                    y_sb = epi.tile([K, D], f32)
                    if STAGE >= 4:
                        nc.vector.tensor_copy(y_sb[:], vlad_acc[:])
                        nc.vector.tensor_copy(y_sb[:, 0:1], a_acc[:])
                    elif STAGE >= 1:
                        nc.vector.tensor_copy(y_sb[:], xt_sb[0:K, 0].rearrange("p a -> p a"))
                    elif STAGE >= 0:
                        nc.vector.memset(y_sb[:], 1.0)
                    else:
                        # touch x_bf so the load isn't dead
                        nc.vector.tensor_copy(y_sb[:], x_bf[0:K, 0, 0:D])
                    nc.sync.dma_start(
                        out=y_dram[n].rearrange("(k d) -> k d", d=D), in_=y_sb[:]
                    )
                    continue
                # ---- epilogue ----
                a_sb = epi.tile([K, 1], f32)
                nc.vector.tensor_copy(a_sb[:], a_acc[:])
                ac = epi.tile([K, D], f32)
                nc.vector.tensor_scalar(
                    out=ac[:], in0=cent_sb[:], scalar1=a_sb[:], scalar2=None,
                    op0=ALU.mult,
                )
                vlad_sb = epi.tile([K, D], f32)
                nc.vector.tensor_tensor(
                    out=vlad_sb[:], in0=vlad_acc[:], in1=ac[:], op=ALU.subtract
                )
                # intra (row) l2 norm
                rss = epi.tile([K, 1], f32)
                nc.scalar.activation(
                    epi_dump[:], vlad_sb[:], FT.Square, accum_out=rss[:]
                )
                rn = epi.tile([K, 1], f32)
                nc.scalar.activation(rn[:], rss[:], FT.Sqrt)
                rinv = epi.tile([K, 1], f32)
                nc.vector.reciprocal(rinv[:], rn[:])
                vlad_n = epi.tile([K, D], f32)
                nc.vector.tensor_scalar(
                    out=vlad_n[:], in0=vlad_sb[:], scalar1=rinv[:], scalar2=None,
                    op0=ALU.mult,
                )
                # global l2 norm
                gss = epi.tile([K, 1], f32)
                nc.scalar.activation(
                    epi_dump[:], vlad_n[:], FT.Square, accum_out=gss[:]
                )
                gtot = epi.tile([K, 1], f32)
                nc.gpsimd.partition_all_reduce(
                    gtot[:], gss[:], channels=K, reduce_op=bass_isa.ReduceOp.add
                )
                gsqrt = epi.tile([K, 1], f32)
                nc.scalar.activation(gsqrt[:], gtot[:], FT.Sqrt)
                ginv = epi.tile([K, 1], f32)
                nc.vector.reciprocal(ginv[:], gsqrt[:])
                y_sb = epi.tile([K, D], f32)
                nc.vector.tensor_scalar(
                    out=y_sb[:], in0=vlad_n[:], scalar1=ginv[:], scalar2=None,
                    op0=ALU.mult,
                )
                nc.sync.dma_start(
                    out=y_dram[n].rearrange("(k d) -> k d", d=D), in_=y_sb[:]
                )
    nc.compile()
    return nc


def _get_nc():
    if "nc" not in _CACHE:
        _CACHE["nc"] = _build()
    return _CACHE["nc"]


def kernel(x, conv_w, centroids):
    from concourse.bass_utils import run_bass_kernel_spmd

    nc = _get_nc()
    xs = np.ascontiguousarray(x.reshape(NCORES, NPER, D, P))
    wt = np.ascontiguousarray(conv_w.T).astype(ml_dtypes.bfloat16)
    ident = np.eye(128, dtype=ml_dtypes.bfloat16)
    cent = np.ascontiguousarray(centroids).astype(np.float32)
    in_maps = [
        {"x": xs[c], "conv_wt": wt, "ident": ident, "cent": cent}
        for c in range(NCORES)
    ]
    res = run_bass_kernel_spmd(
        nc, in_maps, core_ids=list(range(NCORES)), trace=TRACE
    )
    _CACHE["last_result"] = res
    y = np.concatenate([r["y"] for r in res.results], axis=0)
    return y.reshape(N, K * D)


# revision 23
# speedup vs baseline: 1.0060x; 1.0060x over previous
"""NetVLAD Trainium2 kernel (data-parallel over batch across 8 NeuronCores).

Math per image (x: [D=512, P=4096], conv_w: [K=64, D], centroids c: [K, D]):
  xhat = x / ||x||_2(over D, per pixel)
  logitsT[p, k] = sum_d xhat[d, p] * conv_w[k, d]
  a = softmax_k(logitsT)            (|logits| <= ||w_k|| ~ 1.3 -> no max-sub)
  vlad[k, d] = sum_p a[p, k] * xhat[d, p] - (sum_p a[p, k]) * c[k, d]
  out = l2norm_global(l2norm_rows(vlad))

Folding (avoids materializing xhat):
  e[p,k]  = exp(invnorm[p] * raw_logit[p,k])     raw logits from RAW x
  e'[p,k] = e[p,k] * invnorm[p] / S[p]           S = sum_k e
  vlad    = sum_p e'[p,k] * x[p,d]               (matmul2)
  A[k]    = sum_p e'[p,k] * norm[p]              (2nd matmul, rhs = norms)

Implementation notes (bf16 on-chip via SWDGE cast-DMA; Bacc handles the
TRN2 one-wait-per-instruction split via generate_event_semaphores):
  - Per p-chunk the PE issues a closed 4-matmul group (logits, fp32 PSUM)
    then 4 transpose-mode matmuls vs identity (xT chunk, bf16 PSUM).
    The PE hard-faults if it enters transpose mode while any PSUM
    accumulation group is open, so matmul groups are always contiguous
    in PE program order and matmul2 accumulates per-group partials that
    the DVE sums in SBUF.
  - DVE copies xT PSUM->SBUF (bf16 2x mode); ACT does sumsq via
    Square+accum_out. Per-pixel softmax batched per 8-chunk group; all
    per-pixel scalars live on partitions.
  - DMA transposes (xbar) are unusable here: the XPOSE struct takes only
    one sem wait and Tile cannot consolidate its multi-proc deps.
"""

import numpy as np
import ml_dtypes

N, D, HH, WW, K = 32, 512, 64, 64, 64
P = HH * WW            # 4096
NCORES = 8
NPER = N // NCORES     # 4 images per core
DC = D // 128          # 4 d-chunks
PC = P // 128          # 32 p-chunks
GRP = 8                # p-chunks per softmax batch group
NG = PC // GRP

TRACE = False          # test.py sets this for profiled runs
IMGS = NPER            # debug knob: build fewer images
STAGE = 5              # 0 prod,1 +copy/ssq,2 +norms/exp,3 +softmax,4 +mm2,5 full
_CACHE = {}

def _build():
    import concourse.bass as bass
    import concourse.bacc as bacc
    import concourse.tile as tile
    from concourse import mybir
    import concourse.bass_isa as bass_isa

    f32 = mybir.dt.float32
    bf16 = mybir.dt.bfloat16
    FT = mybir.ActivationFunctionType
    ALU = mybir.AluOpType

    nc = bacc.Bacc()
    x_dram = nc.declare_dram_parameter("x", [NPER, D, P], f32, isOutput=False)
    wt_dram = nc.declare_dram_parameter("conv_wt", [D, K], bf16, isOutput=False)
    id_dram = nc.declare_dram_parameter("ident", [128, 128], bf16, isOutput=False)
    c_dram = nc.declare_dram_parameter("cent", [K, D], f32, isOutput=False)
    y_dram = nc.declare_dram_parameter("y", [NPER, K * D], f32, isOutput=True)

    with tile.TileContext(nc) as tc:
        with (
            tc.tile_pool(name="consts", bufs=1) as consts,
            tc.tile_pool(name="xpool", bufs=2) as xpool,
            tc.tile_pool(name="xtpool", bufs=2) as xtpool,
            tc.tile_pool(name="epool", bufs=2) as epool,
            tc.tile_pool(name="stats", bufs=2) as stats,
            tc.tile_pool(name="dumps", bufs=1) as dumps,
            tc.tile_pool(name="epi", bufs=2) as epi,
            tc.tile_pool(name="ps_xt", bufs=2, space="PSUM") as ps_xt,
            tc.tile_pool(name="ps_a", bufs=2, space="PSUM") as ps_a,
            tc.tile_pool(name="ps_log", bufs=2, space="PSUM") as ps_log,
            tc.tile_pool(name="ps_vlad", bufs=2, space="PSUM") as ps_vlad,
        ):
            # ---- constants ----
            wt_sb = consts.tile([128, DC, K], bf16)   # conv_w^T chunked
            nc.gpsimd.dma_start(
                out=wt_sb[:], in_=wt_dram[:].rearrange("(dc dp) k -> dp dc k", dp=128)
            )
            ident = consts.tile([128, 128], bf16)
            nc.gpsimd.dma_start(out=ident[:], in_=id_dram[:])
            cent_sb = consts.tile([K, D], f32)
            nc.gpsimd.dma_start(out=cent_sb[:], in_=c_dram[:])

            act_dump = dumps.tile([128, 512], bf16)
            epi_dump = dumps.tile([K, D], f32)

            for nn in range(IMGS):
                n = nn % NPER
                # ---- load + cast fp32 -> bf16 (SWDGE) ----
                x_bf = xpool.tile([128, DC, P], bf16)
                xn = x_dram[n].rearrange("(dc dp) p -> dp dc p", dp=128)
                nc.gpsimd.dma_start(out=x_bf[:], in_=xn[:])

                sumsq = stats.tile([128, PC], f32)
                invnorm = stats.tile([128, PC], f32)
                norm_bf = stats.tile([128, PC], bf16)
                e_sb = epool.tile([128, PC, K], bf16)
                ep_sb = epool.tile([128, PC, K], bf16)
                S = stats.tile([128, NG, GRP], f32)
                invS = stats.tile([128, PC], f32)
                factor = stats.tile([128, PC, 1], bf16)
                xt_sb = xtpool.tile([128, PC, 512], bf16)
                vlad_acc = epi.tile([K, D], f32)
                a_acc = epi.tile([K, 1], f32)

                # PE wait-absorber: take the x-load wait on a standalone
                # LDWEIGHTS so the first real matmul only carries its
                # pool-recycle wait.
                nc.tensor.ldweights(weights=x_bf[:, 0, 0:128])

                def mm2_group(g):
                    # wait-absorbing weights-load (also warms the PE stream)
                    nc.tensor.ldweights(weights=xt_sb[:, g * GRP + 2, 0:128])
                    # closed per-group PSUM accumulation: the PE may not
                    # switch into transpose mode while a group is open, so
                    # each group's 16 matmuls are contiguous and partials
                    # are accumulated into SBUF by the DVE.
                    vlad_ps = ps_vlad.tile([K, D], f32, tag="vlad_ps")
                    a_ps = ps_a.tile([K, 1], f32, tag="a_ps")
                    for sub in range(GRP):
                        pc = g * GRP + sub
                        nc.tensor.matmul(
                            vlad_ps[:],
                            lhsT=ep_sb[:, pc],
                            rhs=xt_sb[:, pc],
                            start=(sub == 0),
                            stop=(sub == GRP - 1),
                        )
                        nc.tensor.matmul(
                            a_ps[:],
                            lhsT=ep_sb[:, pc],
                            rhs=norm_bf[:, pc:pc + 1],
                            start=(sub == 0),
                            stop=(sub == GRP - 1),
                        )
                    if g == 0:
                        nc.vector.tensor_copy(vlad_acc[:], vlad_ps[:])
                        nc.vector.tensor_copy(a_acc[:], a_ps[:])
                    else:
                        nc.vector.tensor_tensor(
                            out=vlad_acc[:], in0=vlad_acc[:], in1=vlad_ps[:],
                            op=ALU.add,
                        )
                        nc.vector.tensor_tensor(
                            out=a_acc[:], in0=a_acc[:], in1=a_ps[:], op=ALU.add,
                        )

                for g in range(NG if STAGE >= 0 else 0):
                    logT = ps_log.tile([128, GRP, K], f32, tag="logT")
                    for sub in range(GRP):
                        pc = g * GRP + sub
                        # ---- PE: logits + transpose, shared stationary ----
                        xtp = ps_xt.tile([128, 512], bf16, tag="xtp")
                        for dc in range(DC):
                            xchunk = x_bf[:, dc, pc * 128:(pc + 1) * 128]
                            nc.tensor.matmul(
                                logT[:, sub],
                                lhsT=xchunk,
                                rhs=wt_sb[:, dc],
                                start=(dc == 0),
                                stop=(dc == DC - 1),
                            )
                        for dc in range(DC):
                            xchunk = x_bf[:, dc, pc * 128:(pc + 1) * 128]
                            nc.tensor.transpose(
                                xtp[:, dc * 128:(dc + 1) * 128], xchunk, ident[:]
                            )
                        # ---- copy xT PSUM -> SBUF + sumsq (one engine) ----
                        if STAGE < 1:
                            continue
                        nc.vector.tensor_copy(xt_sb[:, pc], xtp[:])
                        nc.scalar.activation(
                            act_dump[:], xtp[:], FT.Square,
                            accum_out=sumsq[:, pc:pc + 1],
                        )

                    # ---- norms for this group ----
                    if STAGE < 2:
                        continue
                    gs = slice(g * GRP, (g + 1) * GRP)
                    norm_g = stats.tile([128, GRP], f32, tag="norm_g")
                    nc.scalar.activation(norm_g[:], sumsq[:, gs], FT.Sqrt)
                    nc.vector.reciprocal(invnorm[:, gs], norm_g[:])
                    nc.vector.tensor_copy(norm_bf[:, gs], norm_g[:])

                    # ---- exp with per-pixel temperature ----
                    for sub in range(GRP):
                        pc = g * GRP + sub
                        nc.scalar.activation(
                            e_sb[:, pc], logT[:, sub], FT.Exp,
                            scale=invnorm[:, pc:pc + 1],
                        )
                    # ---- softmax denominator + folded scale ----
                    if STAGE < 3:
                        continue
                    nc.vector.reduce_sum(
                        S[:, g], e_sb[:, gs], axis=mybir.AxisListType.X
                    )
                    nc.vector.reciprocal(invS[:, gs], S[:, g])
                    nc.vector.tensor_tensor(
                        out=factor[:, gs, 0], in0=invS[:, gs],
                        in1=invnorm[:, gs], op=ALU.mult,
                    )
                    nc.vector.tensor_tensor(
                        out=ep_sb[:, gs],
                        in0=e_sb[:, gs],
                        in1=factor[:, gs].to_broadcast([128, GRP, K]),
                        op=ALU.mult,
                    )
                    # matmul2, software-pipelined one group behind
                    if STAGE >= 4 and g >= 1:
                        mm2_group(g - 1)
                if STAGE >= 4:
                    mm2_group(NG - 1)

                # ---- epilogue ----
                if STAGE < 5:
                    y_sb = epi.tile([K, D], f32)
                    if STAGE >= 4:
                        nc.vector.tensor_copy(y_sb[:], vlad_acc[:])
                        nc.vector.tensor_copy(y_sb[:, 0:1], a_acc[:])
                    elif STAGE >= 1:
                        nc.vector.tensor_copy(y_sb[:], xt_sb[0:K, 0].rearrange("p a -> p a"))
                    elif STAGE >= 0:
                        nc.vector.memset(y_sb[:], 1.0)
                    else:
                        # touch x_bf so the load isn't dead
                        nc.vector.tensor_copy(y_sb[:], x_bf[0:K, 0, 0:D])
                    nc.sync.dma_start(
                        out=y_dram[n].rearrange("(k d) -> k d", d=D), in_=y_sb[:]
                    )
                    continue
                # ---- epilogue ----
                a_sb = epi.tile([K, 1], f32)
                nc.vector.tensor_copy(a_sb[:], a_acc[:])
                ac = epi.tile([K, D], f32)
                nc.vector.tensor_scalar(
                    out=ac[:], in0=cent_sb[:], scalar1=a_sb[:], scalar2=None,
                    op0=ALU.mult,
                )
                vlad_sb = epi.tile([K, D], f32)
                nc.vector.tensor_tensor(
                    out=vlad_sb[:], in0=vlad_acc[:], in1=ac[:], op=ALU.subtract
                )
                # intra (row) l2 norm
                rss = epi.tile([K, 1], f32)
                nc.scalar.activation(
                    epi_dump[:], vlad_sb[:], FT.Square, accum_out=rss[:]
                )
                rn = epi.tile([K, 1], f32)
                nc.scalar.activation(rn[:], rss[:], FT.Sqrt)
                rinv = epi.tile([K, 1], f32)
                nc.vector.reciprocal(rinv[:], rn[:])
                vlad_n = epi.tile([K, D], f32)
                nc.vector.tensor_scalar(
                    out=vlad_n[:], in0=vlad_sb[:], scalar1=rinv[:], scalar2=None,
                    op0=ALU.mult,
                )
                # global l2 norm
                gss = epi.tile([K, 1], f32)
                nc.scalar.activation(
                    epi_dump[:], vlad_n[:], FT.Square, accum_out=gss[:]
                )
                gtot = epi.tile([K, 1], f32)
                nc.gpsimd.partition_all_reduce(
                    gtot[:], gss[:], channels=K, reduce_op=bass_isa.ReduceOp.add
                )
                gsqrt = epi.tile([K, 1], f32)
                nc.scalar.activation(gsqrt[:], gtot[:], FT.Sqrt)
                ginv = epi.tile([K, 1], f32)
                nc.vector.reciprocal(ginv[:], gsqrt[:])
                y_sb = epi.tile([K, D], f32)
                nc.vector.tensor_scalar(
                    out=y_sb[:], in0=vlad_n[:], scalar1=ginv[:], scalar2=None,
                    op0=ALU.mult,
                )
                nc.sync.dma_start(
                    out=y_dram[n].rearrange("(k d) -> k d", d=D), in_=y_sb[:]
                )
    nc.compile()
    return nc


def _get_nc():
    if "nc" not in _CACHE:
        _CACHE["nc"] = _build()
    return _CACHE["nc"]


def kernel(x, conv_w, centroids):
    from concourse.bass_utils import run_bass_kernel_spmd

    nc = _get_nc()
    xs = np.ascontiguousarray(x.reshape(NCORES, NPER, D, P))
    wt = np.ascontiguousarray(conv_w.T).astype(ml_dtypes.bfloat16)
    ident = np.eye(128, dtype=ml_dtypes.bfloat16)
    cent = np.ascontiguousarray(centroids).astype(np.float32)
    in_maps = [
        {"x": xs[c], "conv_wt": wt, "ident": ident, "cent": cent}
        for c in range(NCORES)
    ]
    res = run_bass_kernel_spmd(
        nc, in_maps, core_ids=list(range(NCORES)), trace=TRACE
    )
    _CACHE["last_result"] = res
    y = np.concatenate([r["y"] for r in res.results], axis=0)
    return y.reshape(N, K * D)
